# revision 28
# baseline (speedup 1.0000x reference)
"""Trainium2 Bass kernel for BrainFunctionalConnectivityFeatureExtractionModule.

Math (per batch b, all f32):
    w    = relu(adj + adj_bias)                       (16,16)
    d    = 1/sqrt(sum(w, axis=1) + 1e-5)              (16,)
    lap  = I - d[:,None] * w * d[None,:]              (16,16)
    t1   = lap @ x[b]                                 (16,256)
    cp   = interleave(ones, t1)                       (16,512)
    h    = relu(brelu_bias + cp @ cheb_w)             (16,64)
    out  = h @ fc_w.T + fc_b                          (16,387)

Since the even interleaved lanes of cp are all-ones,
    cp @ cheb_w = t1 @ cheb_w[1::2] + sum(cheb_w[0::2], axis=0)
and by associativity  (lap @ x) @ W1 == lap @ (x @ W1),  so the C=256
contraction runs FIRST (down to H=64 wide), then the tiny 16x16 graph mix:
    y   = x @ W1                      W1 = cheb_w[1::2]   (256,64)
    z   = lap @ y    (per graph; as 128-row blocks: zT = y^T-mix via PE)
    h   = relu(z + bias_h)
    out = h @ fc_w.T + fc_b

This quarters the PSUM->SBUF intermediate traffic vs mixing x itself.

Device mapping: pure data parallel over 8 cores, B=8192 -> 1024 batches/core,
ROWS = 16384 (b,e)-rows per core, in 512-row tiles of 4 x 128-row blocks
(block = 8 complete 16-node graphs).

All HBM I/O is bf16 (gate is 2e-2; bf16 rounding costs ~1e-3).  x is pre-cast
AND pre-transposed on the host into c-on-partitions chunk layout with fully
contiguous multi-KB DMA lines.  Layout/scheduling notes (from NTFF profiles):
  * x loads + output stores ride the Sync HWDGE ring (x in graded chunks
    [1,3,4,8,8,8] tiles: small first chunk so the PE starts ~10us in incl.
    the ~7us engine-boot, 2MB steady chunks for ~97% of HBM rate); consts
    ride the Scalar HWDGE ring so their 4x ~2.3us serial completion cost
    never queues ahead of the first x chunk.  Output stores must NOT issue
    from the Act ring: a store's semaphore wait head-of-line blocks the
    next tile's y-copy in the strict engine FIFO (measured +10us).
  * all matmul stationaries are contiguous 128-col, K=128 bf16 SBUF slices
    so LDWEIGHTS takes the fast-weight-load path AND overlaps the previous
    matmul (measured 53 vs 107+ ns; K=65 stationaries do not FWL).  hT is
    padded to 128 partitions (rows 64=ones for the fc_b fold, 65..127=zeros
    vs zero-padded fcw rows) and stored slot-major via a permuted mix-matmul
    PSUM access pattern, so stage-3 stationaries are contiguous.
  * the hT/pad rows are initialized ONCE on three manually rotated buffers
    (no per-tile memsets in the dependency chain).
  * each 512-f32 PSUM slot is exactly one bank; matmul outputs never
    straddle banks.
  * last output group stores per-tile (4 x 396KB) to cut the drain tail.

Per-tile engine budget: PE ~1.5us (16 MM + 16 LDW), Act ~1.45us (y cast copy
+ 2 out slots + store issue), DVE ~1.46us (fused bias+relu tensor_scalar +
2 out slots), under the ~1.9us/tile DMA floor (21.2 MB/core @ 358 GB/s).
"""

import numpy as np
from contextlib import ExitStack

B, E, C, H, OUT = 8192, 16, 256, 64, 387
NCORES = 8
ROWS = (B // NCORES) * E        # 16384 rows per core
TR = 512                        # rows per tile
NS = TR // 128                  # 128-row blocks per tile
NT = ROWS // TR                 # 32 tiles per core
G = 4                           # tiles per output DMA group
NGR = NT // G                   # 8 output groups per core
XCH = (1, 3, 4, 8, 8, 8)        # x-load chunk sizes in tiles (graded start)
KC = C // 128                   # 2 contraction chunks of 128
OUTP = OUT + 1                  # fc matmul N padded even
HP1 = H + 1                     # fc contraction incl. bias row

_cache = {}


def _build_module():
    import concourse.tile as tile
    from concourse import bacc, mybir

    f32 = mybir.dt.float32
    bf16 = mybir.dt.bfloat16
    Add = mybir.AluOpType.add
    Max = mybir.AluOpType.max

    nc = bacc.Bacc("TRN2", target_bir_lowering=False, debug=False,
                   num_devices=NCORES)

    x_d = [nc.dram_tensor(f"x{i}", (128, nt, KC, TR), bf16,
                          kind="ExternalInput").ap()
           for i, nt in enumerate(XCH)]
    w1_d = nc.dram_tensor("w1", (128, KC, H), bf16, kind="ExternalInput").ap()
    r_d = nc.dram_tensor("r", (128, 128), bf16, kind="ExternalInput").ap()
    bh_d = nc.dram_tensor("bh", (H, 1), f32, kind="ExternalInput").ap()
    fcw_d = nc.dram_tensor("fcw", (128, OUTP), bf16, kind="ExternalInput").ap()
    o_d = nc.dram_tensor("o", (NGR, 128, G, NS, OUT), bf16,
                         kind="ExternalOutput").ap()

    with tile.TileContext(nc) as tc:
        with ExitStack() as ctx:
            consts = ctx.enter_context(tc.tile_pool(name="consts", bufs=1))
            xp = ctx.enter_context(tc.tile_pool(name="xp", bufs=2))
            yp = ctx.enter_context(tc.tile_pool(name="yp", bufs=3))
            hp = ctx.enter_context(tc.tile_pool(name="hp", bufs=3))
            op = ctx.enter_context(tc.tile_pool(name="op", bufs=2))
            otp = ctx.enter_context(tc.tile_pool(name="otp", bufs=2))
            ypp = ctx.enter_context(tc.tile_pool(name="ypp", bufs=2, space="PSUM"))
            zpp = ctx.enter_context(tc.tile_pool(name="zpp", bufs=2, space="PSUM"))
            opp = ctx.enter_context(tc.tile_pool(name="opp", bufs=2, space="PSUM"))

            # x chunk loads lead on the Sync HWDGE ring.  Chunks >= 2 reuse a
            # buffer (bufs=2) so their dma_start carries a WAR wait; emit
            # each at the first tile of the previous chunk so every o-store
            # it may transitively depend on is already ahead of it in the
            # Sync FIFO (deadlock-free, bounded stall).
            starts = [sum(XCH[:i]) for i in range(len(XCH))]
            x_sbs = []

            def load_x_chunk(i):
                x_sb = xp.tile([128, XCH[i], KC, TR], bf16, name=f"x_sb{i}",
                               tag="x_sb")
                nc.sync.dma_start(x_sb, x_d[i])
                x_sbs.append(x_sb)

            load_x_chunk(0)
            load_x_chunk(1)

            # consts ride the Scalar HWDGE ring (parallel to x chunk 0)
            w1_sb = consts.tile([128, KC, H], bf16)
            nc.scalar.dma_start(w1_sb, w1_d)
            r_sb = consts.tile([128, 128], bf16)
            nc.scalar.dma_start(r_sb, r_d)
            bh_sb = consts.tile([H, 1], f32)
            nc.scalar.dma_start(bh_sb, bh_d)
            fcw_sb = consts.tile([128, OUTP], bf16)
            nc.scalar.dma_start(fcw_sb, fcw_d)

            # hT buffers: rows 0..63 rewritten per tile; row 64 = ones (fc_b
            # fold), rows 65..127 = zeros (vs zero-padded fcw rows) -- pad
            # rows initialized once, buffers rotated manually
            hT_bufs = []
            for _ in range(3):
                hT = hp.tile([128, NS, 128], bf16)
                # engine ops need base partition in {0,32,64,96}: zero rows
                # 64..127 first, then overwrite row 64 with ones
                nc.gpsimd.memset(hT[H:128, :, :], 0.0)
                nc.gpsimd.memset(hT[H:HP1, :, :], 1.0)
                hT_bufs.append(hT)

            xch_of_tile = []
            for i, nt in enumerate(XCH):
                xch_of_tile += [(i, j) for j in range(nt)]

            for t in range(NT):
                if len(x_sbs) < len(XCH) and t == starts[len(x_sbs) - 1]:
                    load_x_chunk(len(x_sbs))
                g = t % G
                last_group = t // G == NGR - 1
                if g == 0 and not last_group:
                    o_sb = op.tile([128, G, NS, OUT], bf16)
                if last_group:
                    o_sb = otp.tile([128, 1, NS, OUT], bf16)
                ci, cj = xch_of_tile[t]
                x_sb = x_sbs[ci]

                # stage 1: y[n, h] = sum_c x[n, c] W1[c, h], per 128-row
                # block; lhsT = xT chunk (c on partitions)
                y_ps = ypp.tile([128, NS, H], f32)
                for s in range(NS):
                    for k in range(KC):
                        nc.tensor.matmul(
                            y_ps[:, s, :],
                            lhsT=x_sb[:, cj, k, s * 128:(s + 1) * 128],
                            rhs=w1_sb[:, k, :],
                            start=(k == 0),
                            stop=(k == KC - 1),
                        )
                # mix stationary padded to 128 cols (fast-weight-load);
                # upper 64 cols are never-read garbage
                y_sb = yp.tile([128, NS, 128], bf16)
                nc.scalar.copy(y_sb[:, :, 0:H], y_ps)

                # stage 2: zT[h, l] = sum_j y[j, h] * (I8 (x) lapT)[j, l],
                # scattered slot-major: col l -> [slot l%NS, pos l//NS] so
                # stage-3 stationaries are contiguous
                z_ps = zpp.tile([128, NS, 128], f32)
                for s in range(NS):
                    nc.tensor.matmul(
                        z_ps[:, :, s * 32:(s + 1) * 32]
                        .rearrange("h q p -> h p q"),
                        lhsT=y_sb[:, s, :],
                        rhs=r_sb,
                    )
                hT_sb = hT_bufs[t % 3]
                nc.vector.tensor_scalar(hT_sb[0:H, :, :], z_ps[0:H, :, :],
                                        bh_sb, 0.0, Add, Max)

                # stage 3: slot s covers rows l = NS*p + s
                og = 0 if last_group else g
                for half in range(2):
                    # each 512-f32 slot is exactly one PSUM bank; a matmul
                    # output AP must not straddle banks
                    o_ps = opp.tile([128, 2, 512], f32)
                    for j in range(2):
                        nc.tensor.matmul(
                            o_ps[:, j, 0:OUTP],
                            lhsT=hT_sb[:, half * 2 + j, :],
                            rhs=fcw_sb,
                        )
                    dst = o_sb[:, og, half * 2:half * 2 + 2, :]
                    if half == 0:
                        nc.scalar.copy(dst, o_ps[:, :, 0:OUT])
                    else:
                        nc.vector.tensor_copy(dst, o_ps[:, :, 0:OUT])

                # output stores on the Sync HWDGE ring.  Measured dead ends:
                # Act-ring stores head-of-line block the next y-copy in the
                # strict engine FIFO (+10us); GpSimd SWDGE stores avoid ring
                # serialization but pay Q7 descriptor-gen overhead per store
                # (+4us).  The last group stores per-tile to shorten the
                # drain tail.
                if last_group:
                    nc.sync.dma_start(o_d[t // G][:, g:g + 1], o_sb)
                elif g == G - 1:
                    nc.sync.dma_start(o_d[t // G], o_sb)

    nc.finalize()
    return nc


def _host_prep(adj, adj_bias, cheb_w, brelu_bias, fc_w, fc_b):
    import ml_dtypes

    bf = ml_dtypes.bfloat16
    adj = np.asarray(adj, np.float32)
    w = np.maximum(adj + np.float32(adj_bias.reshape(())), 0.0)
    d = 1.0 / np.sqrt(w.sum(axis=1) + np.float32(1e-5))
    lap = np.eye(E, dtype=np.float32) - d[:, None] * w * d[None, :]

    # r = I_8 (x) lap^T : [j = b*16+jj, n = b*16+i] -> lap[i, jj]
    r = np.kron(np.eye(128 // E, dtype=np.float32), lap.T)

    cheb_w = np.asarray(cheb_w, np.float32)
    w1 = cheb_w[1::2, :].reshape(KC, 128, H).transpose(1, 0, 2)
    bias_h = (cheb_w[0::2, :].sum(axis=0)
              + np.asarray(brelu_bias, np.float32).reshape(H))
    fcw = np.zeros((128, OUTP), np.float32)
    fcw[:H, :OUT] = np.asarray(fc_w, np.float32).T
    fcw[H, :OUT] = np.asarray(fc_b, np.float32)
    return {
        "r": r.astype(bf),
        "w1": np.ascontiguousarray(w1).astype(bf),
        "bh": bias_h.reshape(H, 1).astype(np.float32),
        "fcw": fcw.astype(bf),
    }


def _run(inputs, trace=False, **kw):
    import ml_dtypes
    from concourse import bass_utils

    if "nc" not in _cache:
        _cache["nc"] = _build_module()
    nc = _cache["nc"]

    bf = ml_dtypes.bfloat16
    x = np.asarray(inputs["x"], np.float32)
    weights = _host_prep(inputs["adj"], inputs["adj_bias"], inputs["cheb_w"],
                         inputs["brelu_bias"], inputs["fc_w"], inputs["fc_b"])

    # x chunk i: [t_local, n, k, p_c] -> [p_c, t_local, k, n]: c-contraction
    # on partitions, contiguous multi-KB DMA lines per partition
    shards = x.reshape(NCORES, NT, TR, KC, 128)
    bounds = np.cumsum((0,) + XCH)
    in_maps = []
    for c in range(NCORES):
        m = dict(weights)
        for i in range(len(XCH)):
            m[f"x{i}"] = (shards[c, bounds[i]:bounds[i + 1]]
                          .transpose(3, 0, 2, 1).astype(bf))
        in_maps.append(m)

    res = bass_utils.run_bass_kernel_spmd(
        nc, in_maps, core_ids=list(range(NCORES)), trace=trace, **kw)

    # o[gr, p, g, s, :] holds row (gr*G+g)*TR + p*NS + s
    out = np.concatenate(
        [res.results[c]["o"].transpose(0, 2, 1, 3, 4)
         .reshape(B // NCORES, E, OUT).astype(np.float32)
         for c in range(NCORES)], axis=0)
    return out, res


def kernel(**inputs) -> np.ndarray:
    out, _ = _run(inputs, trace=False)
    return out
